# revision 31
# baseline (speedup 1.0000x reference)
"""DeepseekV2 MoE Trainium2 kernel (8 NeuronCores, expert-parallel).

Strategy
--------
Routing (a [T,16] softmax/top-k control plane, ~0.02% of FLOPs) is computed
on host, exactly replicating the reference semantics. The heavy data plane
runs on 8 cores:

  - Expert parallel: 16 routed experts -> 2 per core ("slot0"/"slot1"),
    bf16 x bf16 -> fp32 PSUM. Host gathers each expert's routed tokens
    (transposed) so the device does dense grouped GEMMs with static shapes.
    Slot capacities C0/C1 are compile-time constants from the actual routing.
  - Shared expert: 2-way split over its intermediate dim (2816 -> 1408 = I,
    exactly 11 k-tiles) x 4-way split over tokens (512 per core), computed
    in fp8-e4m3 with DoubleRow matmuls (2 contraction rows/cycle -> ~2x PE
    throughput, measured 1684ns vs 3453ns per 16-k-tile group). Host
    pre-quantizes x (scale 16) and weights (scale 128), clipped to +-240
    (TRN e4m3 max normal). PSUM scale corrections fold into the silu
    activation (input scale) and a fused DVE (ps*k)*sil op, so fp8 adds
    zero extra engine passes. The shared expert runs FIRST: its weight tile
    (256KB) + token tile (1MB) arrive earliest, minimizing the startup
    bubble.
  - Stage 2 keeps w_down stationary and streams the intermediate
    activations. Outputs come back transposed ([H, tokens]); routed outputs
    are UNSCALED (host applies top-k combine weights and scatter-adds).
  - DMA: weights stream on the sync-engine HWDGE ring in consumption
    order; token tiles go on the gpsimd ring (parallel with the first
    weight tiles, cutting the startup serial-DMA chain); outputs on the
    scalar ring. PSUM->SBUF copies alternate between DVE and ACT.
  - PE warm-up: dummy matmuls on zeroed SBUF keep the tensor engine busy
    during the initial DMA so the HAM clock-gate is released (2.4 GHz)
    when real work arrives, sized to end when the first operands land.
"""

import numpy as np
import ml_dtypes

import concourse.bacc as bacc
import concourse.mybir as mybir
import concourse.tile as tile
from concourse.bass_utils import run_bass_kernel_spmd

BF16 = ml_dtypes.bfloat16
E4NP = ml_dtypes.float8_e4m3
F32 = mybir.dt.float32
BF = mybir.dt.bfloat16
E4 = mybir.dt.float8e4
DR = mybir.MatmulPerfMode.DoubleRow
MUL = mybir.AluOpType.mult

# model dims (hardcoded per problem spec)
H = 2048
I = 1408
E = 16
TOP_K = 4
N_GROUP = 4
TOPK_GROUP = 2
SCALE = 16.0
SI = 2816          # shared expert intermediate (2 * I)
B, S = 1, 2048
T = B * S
N_CORES = 8

KT = H // 128      # 16 k-tiles over hidden dim
IT = I // 128      # 11 i-tiles over routed/shared-half intermediate
MT = 2 * I // 128  # 22 m-tiles over merged gate|up
HT = H // 128      # 16 h-tiles over output hidden dim
SH_TOK = 512       # shared-expert tokens per core (4-way token split)

# fp8 scales for the shared expert (e4m3, clip to +-240 = TRN max normal)
SX = 16.0          # tokens: |x|max 5.1 -> 81
SWG = 128.0        # gate_up weights: sigma 0.022 -> 2.8
SWD = 128.0        # down weights
SA = 8.0           # intermediate silu(g)*u: |a|max ~12.6 -> ~101
K_SIL = 1.0 / (SX * SWG)   # PSUM -> true gate value (silu input scale)
K_UP = SA / (SX * SWG)     # PSUM up-path -> SA * u
K_OUT = 1.0 / (SA * SWD)   # stage-2 PSUM -> true output
FP8_CLIP = 240.0

WARMUP_MMS = 15

_PROGRAM_CACHE = {}
last_run_info = {}


# --------------------------------------------------------------------------
# host routing (exact replication of reference.py semantics)
# --------------------------------------------------------------------------

def _topk_desc_stable(a, k):
    idx = np.argsort(-a, axis=-1, kind="stable")[..., :k]
    return np.take_along_axis(a, idx, axis=-1), idx


def _compute_routing(hidden_states, gate_w):
    x = hidden_states.reshape(-1, H).astype(np.float32)
    logits = x @ gate_w.T.astype(np.float32)                  # [T, E]
    grouped = logits.reshape(T, N_GROUP, E // N_GROUP)
    group_scores = grouped.max(axis=-1)
    _, group_idx = _topk_desc_stable(group_scores, TOPK_GROUP)
    keep = np.zeros((T, N_GROUP), bool)
    rows = np.arange(T)[:, None]
    keep[rows, group_idx] = True
    grouped = np.where(keep[..., None], grouped, np.float32(0.0))
    logits = grouped.reshape(T, E)
    m = logits.max(axis=-1, keepdims=True)
    ex = np.exp(logits - m)
    probs = (ex / ex.sum(axis=-1, keepdims=True)).astype(np.float32)
    topk_w, topk_ids = _topk_desc_stable(probs, TOP_K)
    topk_w = topk_w * np.float32(SCALE)
    combine = np.zeros((T, E), np.float32)
    np.add.at(combine, (rows, topk_ids), topk_w)
    return combine


# --------------------------------------------------------------------------
# device program
# --------------------------------------------------------------------------

def _s1_chunks(total):
    """Stage-1 moving-operand chunks: <=512 wide, 32-aligned, all >=256 so
    per-matmul LDWEIGHTS (107ns) stays hidden under the matmul stream."""
    if total <= 512:
        return [(0, total)]
    n = -(-total // 512)
    base = -(-total // (n * 32)) * 32
    out = []
    o = 0
    while o < total:
        w = min(base, total - o)
        out.append((o, w))
        o += w
    return out


def _build_program(C0, C1):
    """One SPMD program for all 8 cores; C0/C1 = routed slot capacities."""
    ch0 = _s1_chunks(C0)
    ch1 = _s1_chunks(C1)
    # stage-2 column chunks: (source, out-tensor, out-col-base, off, width)
    s2chunks = [(0, 0, 0, o, w) for (o, w) in ch0] + \
               [(1, 0, C0, o, w) for (o, w) in ch1] + \
               [(2, 1, 0, 0, SH_TOK)]

    nc = bacc.Bacc("TRN2", target_bir_lowering=False, debug=False,
                   num_devices=N_CORES)

    def din(name, shape, dt=BF):
        return nc.dram_tensor(name, list(shape), dt, kind="ExternalInput").ap()

    def dout(name, shape, dt=BF):
        return nc.dram_tensor(name, list(shape), dt, kind="ExternalOutput").ap()

    # stage-1 token buffers; shared tokens are fp8
    xt_d = din("xt", [128, KT, SH_TOK], E4)
    xg0a_d = din("xg0a", [128, KT, ch0[0][1]])
    xg0b_d = None
    if len(ch0) > 1:
        xg0b_d = din("xg0b", [128, KT, C0 - ch0[0][1]])
    xg1_d = din("xg1", [128, KT, C1])
    sgu_d = din("sgu", [MT, 128, KT, 128], E4)
    wgu0_d = din("wgu0", [MT, 128, KT, 128])
    wgu1_d = din("wgu1", [MT, 128, KT, 128])
    # stage-2 weights per h-tile: routed [wd0 11][wd1 11] bf16, shared fp8
    wdr_d = din("wdr", [HT, 128, 2 * IT, 128])
    sds_d = din("sds", [HT, 128, IT, 128], E4)
    yrT_d = dout("yrT", [HT, 128, C0 + C1])
    yshT_d = dout("yshT", [HT, 128, SH_TOK])
    youts = [yrT_d, yshT_d]

    with tile.TileContext(nc) as tc:
        with tc.tile_pool(name="persist", bufs=1) as pp, \
             tc.tile_pool(name="wgu_pool", bufs=4) as wgup, \
             tc.tile_pool(name="sgu_pool", bufs=8) as sgup, \
             tc.tile_pool(name="wdr_pool", bufs=3) as wdrp, \
             tc.tile_pool(name="sds_pool", bufs=3) as sdsp, \
             tc.tile_pool(name="out_pool", bufs=6) as op, \
             tc.tile_pool(name="ps1", bufs=4, space="PSUM") as ps1, \
             tc.tile_pool(name="ps2", bufs=4, space="PSUM") as ps2:

        # persistent SBUF state
            xt_sb = pp.tile([128, KT, SH_TOK], E4, name="xt_sb", tag="xt_sb")
            xg0_sb = pp.tile([128, KT, C0], BF, name="xg0_sb", tag="xg0_sb")
            xg1_sb = pp.tile([128, KT, C1], BF, name="xg1_sb", tag="xg1_sb")
            aT0 = pp.tile([128, IT, C0], BF, name="aT0", tag="aT0")
            aT1 = pp.tile([128, IT, C1], BF, name="aT1", tag="aT1")
            sil_s = pp.tile([128, IT, SH_TOK], BF, name="sil_s", tag="sil_s")
            aTs = pp.tile([128, IT, SH_TOK], E4, name="aTs", tag="aTs")
            warm = pp.tile([128, 512], BF, name="warm", tag="warm")

            # ---- PE warm-up: dummy matmuls on zeroed SBUF keep the PE
            # busy through the initial token/weight DMA so the HAM clock
            # gate opens (2.4 GHz) before real work arrives.
            nc.vector.memset(warm[:], 0.0)
            for w in range(WARMUP_MMS):
                psw = ps1.tile([128, 512], F32, name="ps_warm", tag="ps1")
                nc.tensor.matmul(psw[:], warm[:, :128], warm[:],
                                 start=True, stop=True)

            # ---- stage 1 shared expert (fp8 DoubleRow) ----
            # aTs = SA * silu(g) * u, transposed; g/u from sgu^T @ xt
            # shared tokens lead the sync ring: they serialize ahead of the
            # weight tiles at full DMA rate, so the first DoubleRow matmul
            # has both operands ~1.5us after the ring comes up
            # shared tokens split across both rings: the gpsimd half keeps
            # the sync ring's serial prologue short (xt's full 256 packets
            # ahead of sgu4 stalled the PE 4.5us at m~4), while the sync
            # half still arrives early (gpsimd ring alone comes up too
            # late, gapping the warmup seam 4us)
            nc.sync.dma_start(xt_sb[:, :KT // 2], xt_d[:, :KT // 2])
            nc.gpsimd.dma_start(xt_sb[:, KT // 2:], xt_d[:, KT // 2:])
            # the sgu stream is packet-dispatch-bound on one ring (2KB
            # per-partition lines): odd tiles are issued from the SCALAR
            # engine stream, which is paced by the silu chain, so the
            # second ring joins just-in-time instead of contending at t=0.
            # Routed tokens follow on the scalar ring once the weight
            # stream is safely ahead.
            swts = [None] * MT

            def issue_sgu(m, eng):
                swts[m] = sgup.tile([128, KT, 128], E4, name="swt",
                                    tag="sgu")
                eng.dma_start(swts[m][:], sgu_d[m])

            for m in range(6):
                issue_sgu(m, nc.sync)
            for m in range(MT):
                if m + 6 < MT:
                    issue_sgu(m + 6,
                              nc.scalar if (m + 6) % 2 else nc.sync)
                if m == 10:
                    nc.scalar.dma_start(xg0_sb[:, :, :ch0[0][1]], xg0a_d[:])
                elif m == 14:
                    if xg0b_d is not None:
                        nc.scalar.dma_start(xg0_sb[:, :, ch0[0][1]:],
                                            xg0b_d[:])
                elif m == 16:
                    nc.scalar.dma_start(xg1_sb[:], xg1_d[:])
                wt = swts[m]
                ps = ps1.tile([128, SH_TOK], F32, name="ps_sh", tag="ps1")
                for k in range(KT // 2):
                    nc.tensor.matmul(
                        ps[:], wt[:, 2 * k:2 * k + 2],
                        xt_sb[:, 2 * k:2 * k + 2, :],
                        start=(k == 0), stop=(k == KT // 2 - 1),
                        perf_mode=DR)
                if m < IT:
                    nc.scalar.activation(
                        sil_s[:, m], ps[:],
                        mybir.ActivationFunctionType.Silu, scale=K_SIL)
                else:
                    # aTs = (ps * K_UP) * sil  -> SA * u * silu(g), fp8
                    nc.vector.scalar_tensor_tensor(
                        aTs[:, m - IT], ps[:], K_UP, sil_s[:, m - IT],
                        MUL, MUL)

            # ---- stage 1 routed slots (bf16) ----
            def stage1(wgu_d, aT, xg_sb, chunks, side=None):
                for m in range(MT):
                    wt = wgup.tile([128, KT, 128], BF, name="wt", tag="wgu")
                    nc.sync.dma_start(wt[:], wgu_d[m])
                    if side and m in side:
                        side[m]()
                    for (c0, cw) in chunks:
                        ps = ps1.tile([128, cw], F32, name="ps_s1", tag="ps1")
                        for k in range(KT):
                            nc.tensor.matmul(
                                ps[:], wt[:, k], xg_sb[:, k, c0:c0 + cw],
                                start=(k == 0), stop=(k == KT - 1))
                        if m < IT:
                            nc.scalar.activation(
                                aT[:, m, c0:c0 + cw], ps[:],
                                mybir.ActivationFunctionType.Silu)
                        else:
                            nc.vector.tensor_mul(
                                aT[:, m - IT, c0:c0 + cw],
                                aT[:, m - IT, c0:c0 + cw], ps[:])

            # stage-2 weight slices, hoisted so the first two can be
            # prefetched from inside the last stage-1 weight stream
            wdrs = [None] * HT
            sdss = [None] * HT

            def issue_wds(i):
                wdrs[i] = wdrp.tile([128, 2 * IT, 128], BF, name="wsl",
                                    tag="wdr")
                nc.sync.dma_start(wdrs[i][:], wdr_d[i])
                sdss[i] = sdsp.tile([128, IT, 128], E4, name="ssl",
                                    tag="sds")
                nc.sync.dma_start(sdss[i][:], sds_d[i])

            stage1(wgu0_d, aT0, xg0_sb, ch0)
            stage1(wgu1_d, aT1, xg1_sb, ch1,
                   side={17: lambda: issue_wds(0),
                         20: lambda: issue_wds(1)})

            # ---- stage 2 (routed bf16 + shared fp8), w_down stationary ----
            # out^T[h, tokens] accumulated over i-tiles; routed outputs are
            # UNSCALED (combine weights applied on host).
            aTx = [aT0, aT1, aTs]
            for ht in range(HT):
                if wdrs[ht] is None:
                    issue_wds(ht)
                wsl = wdrs[ht]
                ssl = sdss[ht]
                for ci, (src, yo, base, c0, cw) in enumerate(s2chunks):
                    ps = ps2.tile([128, cw], F32, name="ps_s2", tag="ps2")
                    if src < 2:
                        aT = aTx[src]
                        woff = src * IT
                        for it in range(IT):
                            nc.tensor.matmul(
                                ps[:], wsl[:, woff + it],
                                aT[:, it, c0:c0 + cw],
                                start=(it == 0), stop=(it == IT - 1))
                    else:
                        for k in range(IT // 2):
                            nc.tensor.matmul(
                                ps[:], ssl[:, 2 * k:2 * k + 2],
                                aTs[:, 2 * k:2 * k + 2, :],
                                start=(k == 0), stop=False, perf_mode=DR)
                        nc.tensor.matmul(
                            ps[:], ssl[:, IT - 1], aTs[:, IT - 1, :],
                            start=False, stop=True)
                    ot = op.tile([128, 512], BF, name="ot", tag="ot")
                    # alternate copy engine so neither DVE nor ACT gates PE
                    if src == 2:
                        nc.scalar.activation(
                            ot[:, :cw], ps[:],
                            mybir.ActivationFunctionType.Copy, scale=K_OUT)
                    elif ci % 2 == 0:
                        nc.vector.tensor_copy(ot[:, :cw], ps[:])
                    else:
                        nc.scalar.activation(
                            ot[:, :cw], ps[:],
                            mybir.ActivationFunctionType.Copy)
                    # alternate output rings (both paced by the ot-write
                    # dependency) so one ring's packet dispatcher doesn't
                    # accumulate a drain backlog at the end of the kernel
                    oeng = nc.gpsimd if ci % 2 == 0 else nc.scalar
                    oeng.dma_start(
                        youts[yo][ht][:, base + c0:base + c0 + cw],
                        ot[:, :cw])
                if ht + 2 < HT and wdrs[ht + 2] is None:
                    issue_wds(ht + 2)

    nc.finalize()
    return nc


# --------------------------------------------------------------------------
# host data prep
# --------------------------------------------------------------------------

def _tile_wgu(w):  # [H, 2I] -> [MT, 128, KT, 128]
    return np.ascontiguousarray(
        w.reshape(KT, 128, MT, 128).transpose(2, 1, 0, 3))


def _tile_wd_T(w):   # [I, H] -> [HT, 128, IT, 128] (stationary per h-tile)
    return w.reshape(IT, 128, HT, 128).transpose(2, 1, 0, 3)


def _q8(a, s):
    return np.clip(a * np.float32(s), -FP8_CLIP, FP8_CLIP).astype(E4NP)


def kernel(hidden_states, gate_w, w_gate_up, w_down, shared_gate_up,
           shared_down, _trace=False):
    x = np.asarray(hidden_states, np.float32).reshape(T, H)
    combine = _compute_routing(np.asarray(hidden_states, np.float32),
                               np.asarray(gate_w, np.float32))

    idx_lists = [np.nonzero(combine[:, e] != 0.0)[0].astype(np.int64)
                 for e in range(E)]
    counts = np.array([len(ix) for ix in idx_lists])
    order = np.argsort(-counts, kind="stable")
    slot0_experts = [int(order[i]) for i in range(N_CORES)]
    slot1_experts = [int(order[2 * N_CORES - 1 - i]) for i in range(N_CORES)]

    C0 = max(32, int(-(-max(counts[e] for e in slot0_experts) // 32) * 32))
    C1 = max(32, int(-(-max(counts[e] for e in slot1_experts) // 32) * 32))
    ch0 = _s1_chunks(C0)

    key = (C0, C1)
    if key not in _PROGRAM_CACHE:
        _PROGRAM_CACHE[key] = _build_program(C0, C1)
    nc = _PROGRAM_CACHE[key]

    xT16 = np.ascontiguousarray(x.T).astype(BF16)              # [H, T]
    xT8 = _q8(np.ascontiguousarray(x.T), SX)                   # [H, T] fp8
    xT8_t = xT8.reshape(KT, 128, T).transpose(1, 0, 2)         # [128, KT, T]

    wgu16 = np.asarray(w_gate_up, np.float32).astype(BF16)
    wd16 = np.asarray(w_down, np.float32).astype(BF16)
    sgu32 = np.asarray(shared_gate_up, np.float32)
    sdw32 = np.asarray(shared_down, np.float32)

    # shared expert: 2 halves over intermediate dim, fp8, pretiled once
    sgu_t = []
    sds_t = []
    for h in range(2):
        lo = h * I
        sl = np.concatenate([sgu32[:, lo:lo + I], sgu32[:, SI + lo:SI + lo + I]],
                            axis=1)                            # [H, 2816]
        sgu_t.append(_tile_wgu(_q8(sl, SWG)))
        sds_t.append(np.ascontiguousarray(_tile_wd_T(_q8(sdw32[lo:lo + I], SWD))))

    in_maps = []
    meta = []
    for c in range(N_CORES):
        e0, e1 = slot0_experts[c], slot1_experts[c]
        xg0 = np.zeros((128, KT, C0), BF16)
        xg1 = np.zeros((128, KT, C1), BF16)
        for (e, xg) in [(e0, xg0), (e1, xg1)]:
            ix = idx_lists[e]
            g = xT16[:, ix].reshape(KT, 128, len(ix)).transpose(1, 0, 2)
            xg[:, :, :len(ix)] = g

        half, q = c // 4, c % 4
        wdr = np.concatenate([_tile_wd_T(wd16[e0]), _tile_wd_T(wd16[e1])],
                             axis=2)

        im = {
            "xt": np.ascontiguousarray(xT8_t[:, :, q * SH_TOK:(q + 1) * SH_TOK]),
            "xg0a": np.ascontiguousarray(xg0[:, :, :ch0[0][1]]),
            "xg1": xg1,
            "sgu": sgu_t[half],
            "wgu0": _tile_wgu(wgu16[e0]),
            "wgu1": _tile_wgu(wgu16[e1]),
            "wdr": np.ascontiguousarray(wdr),
            "sds": sds_t[half],
        }
        if len(ch0) > 1:
            im["xg0b"] = np.ascontiguousarray(xg0[:, :, ch0[0][1]:])
        in_maps.append(im)
        meta.append((e0, e1))

    res = run_bass_kernel_spmd(nc, in_maps, list(range(N_CORES)),
                               trace=_trace)
    last_run_info["exec_time_ns"] = res.exec_time_ns
    last_run_info["profile_json"] = res.profile_json
    last_run_info["results"] = res

    # ---- host combine (unshard) ----
    out = np.zeros((T, H), np.float32)
    all_idx = []
    all_rows = []
    for c in range(N_CORES):
        # yrT: [HT, 128, C0+C1] -> [C0+C1, H] rows; yshT -> [512, H] partial
        yrT = np.asarray(res.results[c]["yrT"], dtype=BF16)
        yshT = np.asarray(res.results[c]["yshT"], dtype=BF16)
        q = c % 4
        out[q * SH_TOK:(q + 1) * SH_TOK] += \
            yshT.transpose(2, 0, 1).reshape(SH_TOK, H).astype(np.float32)
        yr_full = yrT.transpose(2, 0, 1).reshape(C0 + C1, H).astype(np.float32)
        e0, e1 = meta[c]
        for (e, off) in [(e0, 0), (e1, C0)]:
            ix = idx_lists[e]
            all_idx.append(ix)
            all_rows.append(yr_full[off:off + len(ix)]
                            * combine[ix, e][:, None])
    all_idx = np.concatenate(all_idx)
    all_rows = np.concatenate(all_rows, axis=0)
    if len(all_idx) == TOP_K * T:
        perm = np.argsort(all_idx, kind="stable")
        out += all_rows[perm].reshape(T, TOP_K, H).sum(axis=1)
    else:  # fallback for degenerate routing (a token with <4 experts)
        np.add.at(out, all_idx, all_rows)

    return out.reshape(B, S, H).astype(np.float32)


# revision 32
# speedup vs baseline: 1.0176x; 1.0176x over previous
"""DeepseekV2 MoE Trainium2 kernel (8 NeuronCores, expert-parallel).

Strategy
--------
Routing (a [T,16] softmax/top-k control plane, ~0.02% of FLOPs) is computed
on host, exactly replicating the reference semantics. The heavy data plane
runs on 8 cores:

  - Expert parallel: 16 routed experts -> 2 per core ("slot0"/"slot1"),
    bf16 x bf16 -> fp32 PSUM. Host gathers each expert's routed tokens
    (transposed) so the device does dense grouped GEMMs with static shapes.
    Slot capacities C0/C1 are compile-time constants from the actual routing.
  - Shared expert: 2-way split over its intermediate dim (2816 -> 1408 = I,
    exactly 11 k-tiles) x 4-way split over tokens (512 per core), computed
    in fp8-e4m3 with DoubleRow matmuls (2 contraction rows/cycle -> ~2x PE
    throughput, measured 1684ns vs 3453ns per 16-k-tile group). Host
    pre-quantizes x (scale 16) and weights (scale 128), clipped to +-240
    (TRN e4m3 max normal). PSUM scale corrections fold into the silu
    activation (input scale) and a fused DVE (ps*k)*sil op, so fp8 adds
    zero extra engine passes. The shared expert runs FIRST: its weight tile
    (256KB) + token tile (1MB) arrive earliest, minimizing the startup
    bubble.
  - Stage 2 keeps w_down stationary and streams the intermediate
    activations. Outputs come back transposed ([H, tokens]); routed outputs
    are UNSCALED (host applies top-k combine weights and scatter-adds).
  - DMA: weights stream on the sync-engine HWDGE ring in consumption
    order; token tiles go on the gpsimd ring (parallel with the first
    weight tiles, cutting the startup serial-DMA chain); outputs on the
    scalar ring. PSUM->SBUF copies alternate between DVE and ACT.
  - PE warm-up: dummy matmuls on zeroed SBUF keep the tensor engine busy
    during the initial DMA so the HAM clock-gate is released (2.4 GHz)
    when real work arrives, sized to end when the first operands land.
"""

import numpy as np
import ml_dtypes

import concourse.bacc as bacc
import concourse.mybir as mybir
import concourse.tile as tile
from concourse.bass_utils import run_bass_kernel_spmd

BF16 = ml_dtypes.bfloat16
E4NP = ml_dtypes.float8_e4m3
F32 = mybir.dt.float32
BF = mybir.dt.bfloat16
E4 = mybir.dt.float8e4
DR = mybir.MatmulPerfMode.DoubleRow
MUL = mybir.AluOpType.mult

# model dims (hardcoded per problem spec)
H = 2048
I = 1408
E = 16
TOP_K = 4
N_GROUP = 4
TOPK_GROUP = 2
SCALE = 16.0
SI = 2816          # shared expert intermediate (2 * I)
B, S = 1, 2048
T = B * S
N_CORES = 8

KT = H // 128      # 16 k-tiles over hidden dim
IT = I // 128      # 11 i-tiles over routed/shared-half intermediate
MT = 2 * I // 128  # 22 m-tiles over merged gate|up
HT = H // 128      # 16 h-tiles over output hidden dim
SH_TOK = 512       # shared-expert tokens per core (4-way token split)

# fp8 scales for the shared expert (e4m3, clip to +-240 = TRN max normal)
SX = 16.0          # tokens: |x|max 5.1 -> 81
SWG = 128.0        # gate_up weights: sigma 0.022 -> 2.8
SWD = 128.0        # down weights
SA = 8.0           # intermediate silu(g)*u: |a|max ~12.6 -> ~101
K_SIL = 1.0 / (SX * SWG)   # PSUM -> true gate value (silu input scale)
K_UP = SA / (SX * SWG)     # PSUM up-path -> SA * u
K_OUT = 1.0 / (SA * SWD)   # stage-2 PSUM -> true output
FP8_CLIP = 240.0

WARMUP_MMS = 16

_PROGRAM_CACHE = {}
last_run_info = {}


# --------------------------------------------------------------------------
# host routing (exact replication of reference.py semantics)
# --------------------------------------------------------------------------

def _topk_desc_stable(a, k):
    idx = np.argsort(-a, axis=-1, kind="stable")[..., :k]
    return np.take_along_axis(a, idx, axis=-1), idx


def _compute_routing(hidden_states, gate_w):
    x = hidden_states.reshape(-1, H).astype(np.float32)
    logits = x @ gate_w.T.astype(np.float32)                  # [T, E]
    grouped = logits.reshape(T, N_GROUP, E // N_GROUP)
    group_scores = grouped.max(axis=-1)
    _, group_idx = _topk_desc_stable(group_scores, TOPK_GROUP)
    keep = np.zeros((T, N_GROUP), bool)
    rows = np.arange(T)[:, None]
    keep[rows, group_idx] = True
    grouped = np.where(keep[..., None], grouped, np.float32(0.0))
    logits = grouped.reshape(T, E)
    m = logits.max(axis=-1, keepdims=True)
    ex = np.exp(logits - m)
    probs = (ex / ex.sum(axis=-1, keepdims=True)).astype(np.float32)
    topk_w, topk_ids = _topk_desc_stable(probs, TOP_K)
    topk_w = topk_w * np.float32(SCALE)
    combine = np.zeros((T, E), np.float32)
    np.add.at(combine, (rows, topk_ids), topk_w)
    return combine


# --------------------------------------------------------------------------
# device program
# --------------------------------------------------------------------------

def _s1_chunks(total):
    """Stage-1 moving-operand chunks: <=512 wide, 32-aligned, all >=256 so
    per-matmul LDWEIGHTS (107ns) stays hidden under the matmul stream."""
    if total <= 512:
        return [(0, total)]
    n = -(-total // 512)
    base = -(-total // (n * 32)) * 32
    out = []
    o = 0
    while o < total:
        w = min(base, total - o)
        out.append((o, w))
        o += w
    return out


def _build_program(C0, C1):
    """One SPMD program for all 8 cores; C0/C1 = routed slot capacities."""
    ch0 = _s1_chunks(C0)
    ch1 = _s1_chunks(C1)
    # stage-2 column chunks: (source, out-tensor, out-col-base, off, width)
    s2chunks = [(0, 0, 0, o, w) for (o, w) in ch0] + \
               [(1, 0, C0, o, w) for (o, w) in ch1] + \
               [(2, 1, 0, 0, SH_TOK)]

    nc = bacc.Bacc("TRN2", target_bir_lowering=False, debug=False,
                   num_devices=N_CORES)

    def din(name, shape, dt=BF):
        return nc.dram_tensor(name, list(shape), dt, kind="ExternalInput").ap()

    def dout(name, shape, dt=BF):
        return nc.dram_tensor(name, list(shape), dt, kind="ExternalOutput").ap()

    # stage-1 token buffers; shared tokens are fp8
    xt_d = din("xt", [128, KT, SH_TOK], E4)
    xg0a_d = din("xg0a", [128, KT, ch0[0][1]])
    xg0b_d = None
    if len(ch0) > 1:
        xg0b_d = din("xg0b", [128, KT, C0 - ch0[0][1]])
    xg1_d = din("xg1", [128, KT, C1])
    sgu_d = din("sgu", [MT, 128, KT, 128], E4)
    wgu0_d = din("wgu0", [MT, 128, KT, 128])
    wgu1_d = din("wgu1", [MT, 128, KT, 128])
    # stage-2 weights per h-tile: routed [wd0 11][wd1 11] bf16, shared fp8
    wdr_d = din("wdr", [HT, 128, 2 * IT, 128])
    sds_d = din("sds", [HT, 128, IT, 128], E4)
    yrT_d = dout("yrT", [HT, 128, C0 + C1])
    yshT_d = dout("yshT", [HT, 128, SH_TOK])
    youts = [yrT_d, yshT_d]

    with tile.TileContext(nc) as tc:
        with tc.tile_pool(name="persist", bufs=1) as pp, \
             tc.tile_pool(name="wgu_pool", bufs=4) as wgup, \
             tc.tile_pool(name="sgu_pool", bufs=8) as sgup, \
             tc.tile_pool(name="wdr_pool", bufs=3) as wdrp, \
             tc.tile_pool(name="sds_pool", bufs=3) as sdsp, \
             tc.tile_pool(name="out_pool", bufs=6) as op, \
             tc.tile_pool(name="ps1", bufs=4, space="PSUM") as ps1, \
             tc.tile_pool(name="ps2", bufs=4, space="PSUM") as ps2:

        # persistent SBUF state
            xt_sb = pp.tile([128, KT, SH_TOK], E4, name="xt_sb", tag="xt_sb")
            xg0_sb = pp.tile([128, KT, C0], BF, name="xg0_sb", tag="xg0_sb")
            xg1_sb = pp.tile([128, KT, C1], BF, name="xg1_sb", tag="xg1_sb")
            aT0 = pp.tile([128, IT, C0], BF, name="aT0", tag="aT0")
            aT1 = pp.tile([128, IT, C1], BF, name="aT1", tag="aT1")
            sil_s = pp.tile([128, IT, SH_TOK], BF, name="sil_s", tag="sil_s")
            aTs = pp.tile([128, IT, SH_TOK], E4, name="aTs", tag="aTs")
            warm = pp.tile([128, 512], BF, name="warm", tag="warm")

            # ---- PE warm-up: dummy matmuls on zeroed SBUF keep the PE
            # busy through the initial token/weight DMA so the HAM clock
            # gate opens (2.4 GHz) before real work arrives.
            nc.vector.memset(warm[:], 0.0)
            for w in range(WARMUP_MMS):
                psw = ps1.tile([128, 512], F32, name="ps_warm", tag="ps1")
                nc.tensor.matmul(psw[:], warm[:, :128], warm[:],
                                 start=True, stop=True)

            # ---- stage 1 shared expert (fp8 DoubleRow) ----
            # aTs = SA * silu(g) * u, transposed; g/u from sgu^T @ xt
            # shared tokens lead the sync ring: they serialize ahead of the
            # weight tiles at full DMA rate, so the first DoubleRow matmul
            # has both operands ~1.5us after the ring comes up
            nc.sync.dma_start(xt_sb[:], xt_d[:])
            # the sgu stream is packet-dispatch-bound on one ring (2KB
            # per-partition lines): odd tiles are issued from the SCALAR
            # engine stream, which is paced by the silu chain, so the
            # second ring joins just-in-time instead of contending at t=0.
            # Routed tokens follow on the scalar ring once the weight
            # stream is safely ahead.
            swts = [None] * MT

            def issue_sgu(m, eng):
                swts[m] = sgup.tile([128, KT, 128], E4, name="swt",
                                    tag="sgu")
                eng.dma_start(swts[m][:], sgu_d[m])

            for m in range(6):
                issue_sgu(m, nc.sync)
            for m in range(MT):
                if m + 6 < MT:
                    issue_sgu(m + 6,
                              nc.scalar if (m + 6) % 2 else nc.sync)
                if m == 10:
                    nc.scalar.dma_start(xg0_sb[:, :, :ch0[0][1]], xg0a_d[:])
                elif m == 14:
                    if xg0b_d is not None:
                        nc.scalar.dma_start(xg0_sb[:, :, ch0[0][1]:],
                                            xg0b_d[:])
                elif m == 16:
                    nc.scalar.dma_start(xg1_sb[:], xg1_d[:])
                wt = swts[m]
                ps = ps1.tile([128, SH_TOK], F32, name="ps_sh", tag="ps1")
                for k in range(KT // 2):
                    nc.tensor.matmul(
                        ps[:], wt[:, 2 * k:2 * k + 2],
                        xt_sb[:, 2 * k:2 * k + 2, :],
                        start=(k == 0), stop=(k == KT // 2 - 1),
                        perf_mode=DR)
                if m < IT:
                    nc.scalar.activation(
                        sil_s[:, m], ps[:],
                        mybir.ActivationFunctionType.Silu, scale=K_SIL)
                else:
                    # aTs = (ps * K_UP) * sil  -> SA * u * silu(g), fp8
                    nc.vector.scalar_tensor_tensor(
                        aTs[:, m - IT], ps[:], K_UP, sil_s[:, m - IT],
                        MUL, MUL)

            # ---- stage 1 routed slots (bf16) ----
            def stage1(wgu_d, aT, xg_sb, chunks, side=None):
                for m in range(MT):
                    wt = wgup.tile([128, KT, 128], BF, name="wt", tag="wgu")
                    nc.sync.dma_start(wt[:], wgu_d[m])
                    if side and m in side:
                        side[m]()
                    for (c0, cw) in chunks:
                        ps = ps1.tile([128, cw], F32, name="ps_s1", tag="ps1")
                        for k in range(KT):
                            nc.tensor.matmul(
                                ps[:], wt[:, k], xg_sb[:, k, c0:c0 + cw],
                                start=(k == 0), stop=(k == KT - 1))
                        if m < IT:
                            nc.scalar.activation(
                                aT[:, m, c0:c0 + cw], ps[:],
                                mybir.ActivationFunctionType.Silu)
                        else:
                            nc.vector.tensor_mul(
                                aT[:, m - IT, c0:c0 + cw],
                                aT[:, m - IT, c0:c0 + cw], ps[:])

            # stage-2 weight slices, hoisted so the first two can be
            # prefetched from inside the last stage-1 weight stream
            wdrs = [None] * HT
            sdss = [None] * HT

            def issue_wds(i):
                wdrs[i] = wdrp.tile([128, 2 * IT, 128], BF, name="wsl",
                                    tag="wdr")
                nc.sync.dma_start(wdrs[i][:], wdr_d[i])
                sdss[i] = sdsp.tile([128, IT, 128], E4, name="ssl",
                                    tag="sds")
                nc.sync.dma_start(sdss[i][:], sds_d[i])

            stage1(wgu0_d, aT0, xg0_sb, ch0)
            stage1(wgu1_d, aT1, xg1_sb, ch1,
                   side={17: lambda: issue_wds(0),
                         20: lambda: issue_wds(1)})

            # ---- stage 2 (routed bf16 + shared fp8), w_down stationary ----
            # out^T[h, tokens] accumulated over i-tiles; routed outputs are
            # UNSCALED (combine weights applied on host).
            aTx = [aT0, aT1, aTs]
            for ht in range(HT):
                if wdrs[ht] is None:
                    issue_wds(ht)
                wsl = wdrs[ht]
                ssl = sdss[ht]
                for ci, (src, yo, base, c0, cw) in enumerate(s2chunks):
                    ps = ps2.tile([128, cw], F32, name="ps_s2", tag="ps2")
                    if src < 2:
                        aT = aTx[src]
                        woff = src * IT
                        for it in range(IT):
                            nc.tensor.matmul(
                                ps[:], wsl[:, woff + it],
                                aT[:, it, c0:c0 + cw],
                                start=(it == 0), stop=(it == IT - 1))
                    else:
                        for k in range(IT // 2):
                            nc.tensor.matmul(
                                ps[:], ssl[:, 2 * k:2 * k + 2],
                                aTs[:, 2 * k:2 * k + 2, :],
                                start=(k == 0), stop=False, perf_mode=DR)
                        nc.tensor.matmul(
                            ps[:], ssl[:, IT - 1], aTs[:, IT - 1, :],
                            start=False, stop=True)
                    ot = op.tile([128, 512], BF, name="ot", tag="ot")
                    # alternate copy engine so neither DVE nor ACT gates PE
                    if src == 2:
                        nc.scalar.activation(
                            ot[:, :cw], ps[:],
                            mybir.ActivationFunctionType.Copy, scale=K_OUT)
                    elif ci % 2 == 0:
                        nc.vector.tensor_copy(ot[:, :cw], ps[:])
                    else:
                        nc.scalar.activation(
                            ot[:, :cw], ps[:],
                            mybir.ActivationFunctionType.Copy)
                    nc.scalar.dma_start(
                        youts[yo][ht][:, base + c0:base + c0 + cw],
                        ot[:, :cw])
                if ht + 2 < HT and wdrs[ht + 2] is None:
                    issue_wds(ht + 2)

    nc.finalize()
    return nc


# --------------------------------------------------------------------------
# host data prep
# --------------------------------------------------------------------------

def _tile_wgu(w):  # [H, 2I] -> [MT, 128, KT, 128]
    return np.ascontiguousarray(
        w.reshape(KT, 128, MT, 128).transpose(2, 1, 0, 3))


def _tile_wd_T(w):   # [I, H] -> [HT, 128, IT, 128] (stationary per h-tile)
    return w.reshape(IT, 128, HT, 128).transpose(2, 1, 0, 3)


def _q8(a, s):
    return np.clip(a * np.float32(s), -FP8_CLIP, FP8_CLIP).astype(E4NP)


def kernel(hidden_states, gate_w, w_gate_up, w_down, shared_gate_up,
           shared_down, _trace=False):
    x = np.asarray(hidden_states, np.float32).reshape(T, H)
    combine = _compute_routing(np.asarray(hidden_states, np.float32),
                               np.asarray(gate_w, np.float32))

    idx_lists = [np.nonzero(combine[:, e] != 0.0)[0].astype(np.int64)
                 for e in range(E)]
    counts = np.array([len(ix) for ix in idx_lists])
    order = np.argsort(-counts, kind="stable")
    slot0_experts = [int(order[i]) for i in range(N_CORES)]
    slot1_experts = [int(order[2 * N_CORES - 1 - i]) for i in range(N_CORES)]

    C0 = max(32, int(-(-max(counts[e] for e in slot0_experts) // 32) * 32))
    C1 = max(32, int(-(-max(counts[e] for e in slot1_experts) // 32) * 32))
    ch0 = _s1_chunks(C0)

    key = (C0, C1)
    if key not in _PROGRAM_CACHE:
        _PROGRAM_CACHE[key] = _build_program(C0, C1)
    nc = _PROGRAM_CACHE[key]

    xT16 = np.ascontiguousarray(x.T).astype(BF16)              # [H, T]
    xT8 = _q8(np.ascontiguousarray(x.T), SX)                   # [H, T] fp8
    xT8_t = xT8.reshape(KT, 128, T).transpose(1, 0, 2)         # [128, KT, T]

    wgu16 = np.asarray(w_gate_up, np.float32).astype(BF16)
    wd16 = np.asarray(w_down, np.float32).astype(BF16)
    sgu32 = np.asarray(shared_gate_up, np.float32)
    sdw32 = np.asarray(shared_down, np.float32)

    # shared expert: 2 halves over intermediate dim, fp8, pretiled once
    sgu_t = []
    sds_t = []
    for h in range(2):
        lo = h * I
        sl = np.concatenate([sgu32[:, lo:lo + I], sgu32[:, SI + lo:SI + lo + I]],
                            axis=1)                            # [H, 2816]
        sgu_t.append(_tile_wgu(_q8(sl, SWG)))
        sds_t.append(np.ascontiguousarray(_tile_wd_T(_q8(sdw32[lo:lo + I], SWD))))

    in_maps = []
    meta = []
    for c in range(N_CORES):
        e0, e1 = slot0_experts[c], slot1_experts[c]
        xg0 = np.zeros((128, KT, C0), BF16)
        xg1 = np.zeros((128, KT, C1), BF16)
        for (e, xg) in [(e0, xg0), (e1, xg1)]:
            ix = idx_lists[e]
            g = xT16[:, ix].reshape(KT, 128, len(ix)).transpose(1, 0, 2)
            xg[:, :, :len(ix)] = g

        half, q = c // 4, c % 4
        wdr = np.concatenate([_tile_wd_T(wd16[e0]), _tile_wd_T(wd16[e1])],
                             axis=2)

        im = {
            "xt": np.ascontiguousarray(xT8_t[:, :, q * SH_TOK:(q + 1) * SH_TOK]),
            "xg0a": np.ascontiguousarray(xg0[:, :, :ch0[0][1]]),
            "xg1": xg1,
            "sgu": sgu_t[half],
            "wgu0": _tile_wgu(wgu16[e0]),
            "wgu1": _tile_wgu(wgu16[e1]),
            "wdr": np.ascontiguousarray(wdr),
            "sds": sds_t[half],
        }
        if len(ch0) > 1:
            im["xg0b"] = np.ascontiguousarray(xg0[:, :, ch0[0][1]:])
        in_maps.append(im)
        meta.append((e0, e1))

    res = run_bass_kernel_spmd(nc, in_maps, list(range(N_CORES)),
                               trace=_trace)
    last_run_info["exec_time_ns"] = res.exec_time_ns
    last_run_info["profile_json"] = res.profile_json
    last_run_info["results"] = res

    # ---- host combine (unshard) ----
    out = np.zeros((T, H), np.float32)
    all_idx = []
    all_rows = []
    for c in range(N_CORES):
        # yrT: [HT, 128, C0+C1] -> [C0+C1, H] rows; yshT -> [512, H] partial
        yrT = np.asarray(res.results[c]["yrT"], dtype=BF16)
        yshT = np.asarray(res.results[c]["yshT"], dtype=BF16)
        q = c % 4
        out[q * SH_TOK:(q + 1) * SH_TOK] += \
            yshT.transpose(2, 0, 1).reshape(SH_TOK, H).astype(np.float32)
        yr_full = yrT.transpose(2, 0, 1).reshape(C0 + C1, H).astype(np.float32)
        e0, e1 = meta[c]
        for (e, off) in [(e0, 0), (e1, C0)]:
            ix = idx_lists[e]
            all_idx.append(ix)
            all_rows.append(yr_full[off:off + len(ix)]
                            * combine[ix, e][:, None])
    all_idx = np.concatenate(all_idx)
    all_rows = np.concatenate(all_rows, axis=0)
    if len(all_idx) == TOP_K * T:
        perm = np.argsort(all_idx, kind="stable")
        out += all_rows[perm].reshape(T, TOP_K, H).sum(axis=1)
    else:  # fallback for degenerate routing (a token with <4 experts)
        np.add.at(out, all_idx, all_rows)

    return out.reshape(B, S, H).astype(np.float32)


# revision 35
# speedup vs baseline: 1.0193x; 1.0017x over previous
"""DeepseekV2 MoE Trainium2 kernel (8 NeuronCores, expert-parallel).

Strategy
--------
Routing (a [T,16] softmax/top-k control plane, ~0.02% of FLOPs) is computed
on host, exactly replicating the reference semantics. The heavy data plane
runs on 8 cores:

  - Expert parallel: 16 routed experts -> 2 per core ("slot0"/"slot1"),
    bf16 x bf16 -> fp32 PSUM. Host gathers each expert's routed tokens
    (transposed) so the device does dense grouped GEMMs with static shapes.
    Slot capacities C0/C1 are compile-time constants from the actual routing.
  - Shared expert: 2-way split over its intermediate dim (2816 -> 1408 = I,
    exactly 11 k-tiles) x 4-way split over tokens (512 per core), computed
    in fp8-e4m3 with DoubleRow matmuls (2 contraction rows/cycle -> ~2x PE
    throughput, measured 1684ns vs 3453ns per 16-k-tile group). Host
    pre-quantizes x (scale 16) and weights (scale 128), clipped to +-240
    (TRN e4m3 max normal). PSUM scale corrections fold into the silu
    activation (input scale) and a fused DVE (ps*k)*sil op, so fp8 adds
    zero extra engine passes. The shared expert runs FIRST: its weight tile
    (256KB) + token tile (1MB) arrive earliest, minimizing the startup
    bubble.
  - Stage 2 keeps w_down stationary and streams the intermediate
    activations. Outputs come back transposed ([H, tokens]); routed outputs
    are UNSCALED (host applies top-k combine weights and scatter-adds).
  - DMA: weights stream on the sync-engine HWDGE ring in consumption
    order; token tiles go on the gpsimd ring (parallel with the first
    weight tiles, cutting the startup serial-DMA chain); outputs on the
    scalar ring. PSUM->SBUF copies alternate between DVE and ACT.
  - PE warm-up: dummy matmuls on zeroed SBUF keep the tensor engine busy
    during the initial DMA so the HAM clock-gate is released (2.4 GHz)
    when real work arrives, sized to end when the first operands land.
"""

import numpy as np
import ml_dtypes

import concourse.bacc as bacc
import concourse.mybir as mybir
import concourse.tile as tile
from concourse.bass_utils import run_bass_kernel_spmd

BF16 = ml_dtypes.bfloat16
E4NP = ml_dtypes.float8_e4m3
F32 = mybir.dt.float32
BF = mybir.dt.bfloat16
E4 = mybir.dt.float8e4
DR = mybir.MatmulPerfMode.DoubleRow
MUL = mybir.AluOpType.mult

# model dims (hardcoded per problem spec)
H = 2048
I = 1408
E = 16
TOP_K = 4
N_GROUP = 4
TOPK_GROUP = 2
SCALE = 16.0
SI = 2816          # shared expert intermediate (2 * I)
B, S = 1, 2048
T = B * S
N_CORES = 8

KT = H // 128      # 16 k-tiles over hidden dim
IT = I // 128      # 11 i-tiles over routed/shared-half intermediate
MT = 2 * I // 128  # 22 m-tiles over merged gate|up
HT = H // 128      # 16 h-tiles over output hidden dim
SH_TOK = 512       # shared-expert tokens per core (4-way token split)

# fp8 scales for the shared expert (e4m3, clip to +-240 = TRN max normal)
SX = 16.0          # tokens: |x|max 5.1 -> 81
SWG = 128.0        # gate_up weights: sigma 0.022 -> 2.8
SWD = 128.0        # down weights
SA = 8.0           # intermediate silu(g)*u: |a|max ~12.6 -> ~101
K_SIL = 1.0 / (SX * SWG)   # PSUM -> true gate value (silu input scale)
K_UP = SA / (SX * SWG)     # PSUM up-path -> SA * u
K_OUT = 1.0 / (SA * SWD)   # stage-2 PSUM -> true output
FP8_CLIP = 240.0

WARMUP_MMS = 16

_PROGRAM_CACHE = {}
last_run_info = {}


# --------------------------------------------------------------------------
# host routing (exact replication of reference.py semantics)
# --------------------------------------------------------------------------

def _topk_desc_stable(a, k):
    idx = np.argsort(-a, axis=-1, kind="stable")[..., :k]
    return np.take_along_axis(a, idx, axis=-1), idx


def _compute_routing(hidden_states, gate_w):
    x = hidden_states.reshape(-1, H).astype(np.float32)
    logits = x @ gate_w.T.astype(np.float32)                  # [T, E]
    grouped = logits.reshape(T, N_GROUP, E // N_GROUP)
    group_scores = grouped.max(axis=-1)
    _, group_idx = _topk_desc_stable(group_scores, TOPK_GROUP)
    keep = np.zeros((T, N_GROUP), bool)
    rows = np.arange(T)[:, None]
    keep[rows, group_idx] = True
    grouped = np.where(keep[..., None], grouped, np.float32(0.0))
    logits = grouped.reshape(T, E)
    m = logits.max(axis=-1, keepdims=True)
    ex = np.exp(logits - m)
    probs = (ex / ex.sum(axis=-1, keepdims=True)).astype(np.float32)
    topk_w, topk_ids = _topk_desc_stable(probs, TOP_K)
    topk_w = topk_w * np.float32(SCALE)
    combine = np.zeros((T, E), np.float32)
    np.add.at(combine, (rows, topk_ids), topk_w)
    return combine


# --------------------------------------------------------------------------
# device program
# --------------------------------------------------------------------------

def _s1_chunks(total):
    """Stage-1 moving-operand chunks: <=512 wide, 32-aligned, all >=256 so
    per-matmul LDWEIGHTS (107ns) stays hidden under the matmul stream."""
    if total <= 512:
        return [(0, total)]
    n = -(-total // 512)
    base = -(-total // (n * 32)) * 32
    out = []
    o = 0
    while o < total:
        w = min(base, total - o)
        out.append((o, w))
        o += w
    return out


def _build_program(C0, C1):
    """One SPMD program for all 8 cores; C0/C1 = routed slot capacities."""
    ch0 = _s1_chunks(C0)
    ch1 = _s1_chunks(C1)
    # stage-2 column chunks: (source, out-tensor, out-col-base, off, width)
    s2chunks = [(0, 0, 0, o, w) for (o, w) in ch0] + \
               [(1, 0, C0, o, w) for (o, w) in ch1] + \
               [(2, 1, 0, 0, SH_TOK)]

    nc = bacc.Bacc("TRN2", target_bir_lowering=False, debug=False,
                   num_devices=N_CORES)

    def din(name, shape, dt=BF):
        return nc.dram_tensor(name, list(shape), dt, kind="ExternalInput").ap()

    def dout(name, shape, dt=BF):
        return nc.dram_tensor(name, list(shape), dt, kind="ExternalOutput").ap()

    # stage-1 token buffers; shared tokens are fp8
    xt_d = din("xt", [128, KT, SH_TOK], E4)
    xg0a_d = din("xg0a", [128, KT, ch0[0][1]])
    xg0b_d = None
    if len(ch0) > 1:
        xg0b_d = din("xg0b", [128, KT, C0 - ch0[0][1]])
    xg1_d = din("xg1", [128, KT, C1])
    sgu_d = din("sgu", [MT, 128, KT, 128], E4)
    wgu0_d = din("wgu0", [MT, 128, KT, 128])
    wgu1_d = din("wgu1", [MT, 128, KT, 128])
    # stage-2 weights per h-tile: routed [wd0 11][wd1 11] bf16, shared fp8
    wdr_d = din("wdr", [HT, 128, 2 * IT, 128])
    sds_d = din("sds", [HT, 128, IT, 128], E4)
    yrT_d = dout("yrT", [HT, 128, C0 + C1])
    yshT_d = dout("yshT", [HT, 128, SH_TOK])
    youts = [yrT_d, yshT_d]

    with tile.TileContext(nc) as tc:
        with tc.tile_pool(name="persist", bufs=1) as pp, \
             tc.tile_pool(name="wgu_pool", bufs=4) as wgup, \
             tc.tile_pool(name="sgu_pool", bufs=8) as sgup, \
             tc.tile_pool(name="wdr_pool", bufs=4) as wdrp, \
             tc.tile_pool(name="sds_pool", bufs=4) as sdsp, \
             tc.tile_pool(name="out_pool", bufs=6) as op, \
             tc.tile_pool(name="ps1", bufs=4, space="PSUM") as ps1, \
             tc.tile_pool(name="ps2", bufs=4, space="PSUM") as ps2:

        # persistent SBUF state
            xt_sb = pp.tile([128, KT, SH_TOK], E4, name="xt_sb", tag="xt_sb")
            xg0_sb = pp.tile([128, KT, C0], BF, name="xg0_sb", tag="xg0_sb")
            xg1_sb = pp.tile([128, KT, C1], BF, name="xg1_sb", tag="xg1_sb")
            aT0 = pp.tile([128, IT, C0], BF, name="aT0", tag="aT0")
            aT1 = pp.tile([128, IT, C1], BF, name="aT1", tag="aT1")
            sil_s = pp.tile([128, IT, SH_TOK], BF, name="sil_s", tag="sil_s")
            aTs = pp.tile([128, IT, SH_TOK], E4, name="aTs", tag="aTs")
            warm = pp.tile([128, 512], BF, name="warm", tag="warm")

            # ---- PE warm-up: dummy matmuls on zeroed SBUF keep the PE
            # busy through the initial token/weight DMA so the HAM clock
            # gate opens (2.4 GHz) before real work arrives.
            nc.vector.memset(warm[:], 0.0)
            for w in range(WARMUP_MMS):
                psw = ps1.tile([128, 512], F32, name="ps_warm", tag="ps1")
                nc.tensor.matmul(psw[:], warm[:, :128], warm[:],
                                 start=True, stop=True)

            # ---- stage 1 shared expert (fp8 DoubleRow) ----
            # aTs = SA * silu(g) * u, transposed; g/u from sgu^T @ xt
            # shared tokens lead the sync ring: they serialize ahead of the
            # weight tiles at full DMA rate, so the first DoubleRow matmul
            # has both operands ~1.5us after the ring comes up
            nc.sync.dma_start(xt_sb[:], xt_d[:])
            # the sgu stream is packet-dispatch-bound on one ring (2KB
            # per-partition lines): odd tiles are issued from the SCALAR
            # engine stream, which is paced by the silu chain, so the
            # second ring joins just-in-time instead of contending at t=0.
            # Routed tokens follow on the scalar ring once the weight
            # stream is safely ahead.
            swts = [None] * MT

            def issue_sgu(m, eng):
                swts[m] = sgup.tile([128, KT, 128], E4, name="swt",
                                    tag="sgu")
                eng.dma_start(swts[m][:], sgu_d[m])

            for m in range(6):
                issue_sgu(m, nc.sync)
            for m in range(MT):
                if m + 6 < MT:
                    issue_sgu(m + 6,
                              nc.scalar if (m + 6) % 2 else nc.sync)
                if m == 10:
                    nc.scalar.dma_start(xg0_sb[:, :, :ch0[0][1]], xg0a_d[:])
                elif m == 14:
                    if xg0b_d is not None:
                        nc.scalar.dma_start(xg0_sb[:, :, ch0[0][1]:],
                                            xg0b_d[:])
                elif m == 16:
                    nc.scalar.dma_start(xg1_sb[:], xg1_d[:])
                wt = swts[m]
                ps = ps1.tile([128, SH_TOK], F32, name="ps_sh", tag="ps1")
                for k in range(KT // 2):
                    nc.tensor.matmul(
                        ps[:], wt[:, 2 * k:2 * k + 2],
                        xt_sb[:, 2 * k:2 * k + 2, :],
                        start=(k == 0), stop=(k == KT // 2 - 1),
                        perf_mode=DR)
                if m < IT:
                    nc.scalar.activation(
                        sil_s[:, m], ps[:],
                        mybir.ActivationFunctionType.Silu, scale=K_SIL)
                else:
                    # aTs = (ps * K_UP) * sil  -> SA * u * silu(g), fp8
                    nc.vector.scalar_tensor_tensor(
                        aTs[:, m - IT], ps[:], K_UP, sil_s[:, m - IT],
                        MUL, MUL)

            # ---- stage 1 routed slots (bf16) ----
            def stage1(wgu_d, aT, xg_sb, chunks, side=None):
                for m in range(MT):
                    wt = wgup.tile([128, KT, 128], BF, name="wt", tag="wgu")
                    nc.sync.dma_start(wt[:], wgu_d[m])
                    if side and m in side:
                        side[m]()
                    for (c0, cw) in chunks:
                        ps = ps1.tile([128, cw], F32, name="ps_s1", tag="ps1")
                        for k in range(KT):
                            nc.tensor.matmul(
                                ps[:], wt[:, k], xg_sb[:, k, c0:c0 + cw],
                                start=(k == 0), stop=(k == KT - 1))
                        if m < IT:
                            nc.scalar.activation(
                                aT[:, m, c0:c0 + cw], ps[:],
                                mybir.ActivationFunctionType.Silu)
                        else:
                            nc.vector.tensor_mul(
                                aT[:, m - IT, c0:c0 + cw],
                                aT[:, m - IT, c0:c0 + cw], ps[:])

            # stage-2 weight slices, hoisted so the first two can be
            # prefetched from inside the last stage-1 weight stream
            wdrs = [None] * HT
            sdss = [None] * HT

            def issue_wds(i):
                wdrs[i] = wdrp.tile([128, 2 * IT, 128], BF, name="wsl",
                                    tag="wdr")
                nc.sync.dma_start(wdrs[i][:], wdr_d[i])
                sdss[i] = sdsp.tile([128, IT, 128], E4, name="ssl",
                                    tag="sds")
                nc.sync.dma_start(sdss[i][:], sds_d[i])

            stage1(wgu0_d, aT0, xg0_sb, ch0)
            stage1(wgu1_d, aT1, xg1_sb, ch1,
                   side={14: lambda: issue_wds(0),
                         17: lambda: issue_wds(1),
                         20: lambda: issue_wds(2)})

            # ---- stage 2 (routed bf16 + shared fp8), w_down stationary ----
            # out^T[h, tokens] accumulated over i-tiles; routed outputs are
            # UNSCALED (combine weights applied on host).
            aTx = [aT0, aT1, aTs]
            for ht in range(HT):
                if wdrs[ht] is None:
                    issue_wds(ht)
                wsl = wdrs[ht]
                ssl = sdss[ht]
                for ci, (src, yo, base, c0, cw) in enumerate(s2chunks):
                    ps = ps2.tile([128, cw], F32, name="ps_s2", tag="ps2")
                    if src < 2:
                        aT = aTx[src]
                        woff = src * IT
                        for it in range(IT):
                            nc.tensor.matmul(
                                ps[:], wsl[:, woff + it],
                                aT[:, it, c0:c0 + cw],
                                start=(it == 0), stop=(it == IT - 1))
                    else:
                        for k in range(IT // 2):
                            nc.tensor.matmul(
                                ps[:], ssl[:, 2 * k:2 * k + 2],
                                aTs[:, 2 * k:2 * k + 2, :],
                                start=(k == 0), stop=False, perf_mode=DR)
                        nc.tensor.matmul(
                            ps[:], ssl[:, IT - 1], aTs[:, IT - 1, :],
                            start=False, stop=True)
                    ot = op.tile([128, 512], BF, name="ot", tag="ot")
                    # alternate copy engine so neither DVE nor ACT gates PE
                    if src == 2:
                        nc.scalar.activation(
                            ot[:, :cw], ps[:],
                            mybir.ActivationFunctionType.Copy, scale=K_OUT)
                    elif ci % 2 == 0:
                        nc.vector.tensor_copy(ot[:, :cw], ps[:])
                    else:
                        nc.scalar.activation(
                            ot[:, :cw], ps[:],
                            mybir.ActivationFunctionType.Copy)
                    nc.scalar.dma_start(
                        youts[yo][ht][:, base + c0:base + c0 + cw],
                        ot[:, :cw])
                if ht + 3 < HT and wdrs[ht + 3] is None:
                    issue_wds(ht + 3)

    nc.finalize()
    return nc


# --------------------------------------------------------------------------
# host data prep
# --------------------------------------------------------------------------

def _tile_wgu(w):  # [H, 2I] -> [MT, 128, KT, 128]
    return np.ascontiguousarray(
        w.reshape(KT, 128, MT, 128).transpose(2, 1, 0, 3))


def _tile_wd_T(w):   # [I, H] -> [HT, 128, IT, 128] (stationary per h-tile)
    return w.reshape(IT, 128, HT, 128).transpose(2, 1, 0, 3)


def _q8(a, s):
    return np.clip(a * np.float32(s), -FP8_CLIP, FP8_CLIP).astype(E4NP)


def kernel(hidden_states, gate_w, w_gate_up, w_down, shared_gate_up,
           shared_down, _trace=False):
    x = np.asarray(hidden_states, np.float32).reshape(T, H)
    combine = _compute_routing(np.asarray(hidden_states, np.float32),
                               np.asarray(gate_w, np.float32))

    idx_lists = [np.nonzero(combine[:, e] != 0.0)[0].astype(np.int64)
                 for e in range(E)]
    counts = np.array([len(ix) for ix in idx_lists])
    order = np.argsort(-counts, kind="stable")
    slot0_experts = [int(order[i]) for i in range(N_CORES)]
    slot1_experts = [int(order[2 * N_CORES - 1 - i]) for i in range(N_CORES)]

    C0 = max(32, int(-(-max(counts[e] for e in slot0_experts) // 32) * 32))
    C1 = max(32, int(-(-max(counts[e] for e in slot1_experts) // 32) * 32))
    ch0 = _s1_chunks(C0)

    key = (C0, C1)
    if key not in _PROGRAM_CACHE:
        _PROGRAM_CACHE[key] = _build_program(C0, C1)
    nc = _PROGRAM_CACHE[key]

    xT16 = np.ascontiguousarray(x.T).astype(BF16)              # [H, T]
    xT8 = _q8(np.ascontiguousarray(x.T), SX)                   # [H, T] fp8
    xT8_t = xT8.reshape(KT, 128, T).transpose(1, 0, 2)         # [128, KT, T]

    wgu16 = np.asarray(w_gate_up, np.float32).astype(BF16)
    wd16 = np.asarray(w_down, np.float32).astype(BF16)
    sgu32 = np.asarray(shared_gate_up, np.float32)
    sdw32 = np.asarray(shared_down, np.float32)

    # shared expert: 2 halves over intermediate dim, fp8, pretiled once
    sgu_t = []
    sds_t = []
    for h in range(2):
        lo = h * I
        sl = np.concatenate([sgu32[:, lo:lo + I], sgu32[:, SI + lo:SI + lo + I]],
                            axis=1)                            # [H, 2816]
        sgu_t.append(_tile_wgu(_q8(sl, SWG)))
        sds_t.append(np.ascontiguousarray(_tile_wd_T(_q8(sdw32[lo:lo + I], SWD))))

    in_maps = []
    meta = []
    for c in range(N_CORES):
        e0, e1 = slot0_experts[c], slot1_experts[c]
        xg0 = np.zeros((128, KT, C0), BF16)
        xg1 = np.zeros((128, KT, C1), BF16)
        for (e, xg) in [(e0, xg0), (e1, xg1)]:
            ix = idx_lists[e]
            g = xT16[:, ix].reshape(KT, 128, len(ix)).transpose(1, 0, 2)
            xg[:, :, :len(ix)] = g

        half, q = c // 4, c % 4
        wdr = np.concatenate([_tile_wd_T(wd16[e0]), _tile_wd_T(wd16[e1])],
                             axis=2)

        im = {
            "xt": np.ascontiguousarray(xT8_t[:, :, q * SH_TOK:(q + 1) * SH_TOK]),
            "xg0a": np.ascontiguousarray(xg0[:, :, :ch0[0][1]]),
            "xg1": xg1,
            "sgu": sgu_t[half],
            "wgu0": _tile_wgu(wgu16[e0]),
            "wgu1": _tile_wgu(wgu16[e1]),
            "wdr": np.ascontiguousarray(wdr),
            "sds": sds_t[half],
        }
        if len(ch0) > 1:
            im["xg0b"] = np.ascontiguousarray(xg0[:, :, ch0[0][1]:])
        in_maps.append(im)
        meta.append((e0, e1))

    res = run_bass_kernel_spmd(nc, in_maps, list(range(N_CORES)),
                               trace=_trace)
    last_run_info["exec_time_ns"] = res.exec_time_ns
    last_run_info["profile_json"] = res.profile_json
    last_run_info["results"] = res

    # ---- host combine (unshard) ----
    out = np.zeros((T, H), np.float32)
    all_idx = []
    all_rows = []
    for c in range(N_CORES):
        # yrT: [HT, 128, C0+C1] -> [C0+C1, H] rows; yshT -> [512, H] partial
        yrT = np.asarray(res.results[c]["yrT"], dtype=BF16)
        yshT = np.asarray(res.results[c]["yshT"], dtype=BF16)
        q = c % 4
        out[q * SH_TOK:(q + 1) * SH_TOK] += \
            yshT.transpose(2, 0, 1).reshape(SH_TOK, H).astype(np.float32)
        yr_full = yrT.transpose(2, 0, 1).reshape(C0 + C1, H).astype(np.float32)
        e0, e1 = meta[c]
        for (e, off) in [(e0, 0), (e1, C0)]:
            ix = idx_lists[e]
            all_idx.append(ix)
            all_rows.append(yr_full[off:off + len(ix)]
                            * combine[ix, e][:, None])
    all_idx = np.concatenate(all_idx)
    all_rows = np.concatenate(all_rows, axis=0)
    if len(all_idx) == TOP_K * T:
        perm = np.argsort(all_idx, kind="stable")
        out += all_rows[perm].reshape(T, TOP_K, H).sum(axis=1)
    else:  # fallback for degenerate routing (a token with <4 experts)
        np.add.at(out, all_idx, all_rows)

    return out.reshape(B, S, H).astype(np.float32)
